# revision 7
# baseline (speedup 1.0000x reference)
"""Causal self-attention (nn_CausalAttention), TP-sharded Bass kernel
for 8 Trainium2 NeuronCores.

Contract: kernel(x, w_qkv, w_out) takes the FULL fp32 inputs
(x [1,4096,1024], w_qkv [3072,1024], w_out [1024,1024]) and returns the
FULL fp32 output [1,4096,1024].

Sharding: tensor-parallel over heads -- 16 heads / 8 cores = 2 heads per
core. qkv weights are column-sharded, w_out row-sharded; each core
computes a full-shape partial output (transposed, f16), the host sums
the 8 partials in f32 and transposes back.

Per-core kernel (all matmul operands fp16, PSUM accumulation fp32):
  - proj: qT/kT/vT [128,4096] dim-major; v DMA-XBAR-transposed into
    natural-layout v_aug [pos, dim] with a ones column appended (so the
    PV matmul also emits the softmax denominators).  Projections of
    chunk j+2 stream through a dedicated PSUM bank interleaved into
    chunk j's attention jobs.
  - scoresT[skv,sq]: the two heads' K=64 matmuls sit on PE row groups
    0/64 (tile_position from base partitions) and are emitted
    back-to-back, so they execute concurrently -- a pair costs about
    one 512-col matmul instead of two.
  - exp on ACT with the 1/sqrt(hd) scale fused, one [128,2,512]
    instruction per skv tile covering both heads; every 4th
    below-diagonal tile instead uses a Schraudolph f16-bit-trick exp on
    the (otherwise underused) DVE: bits = rne(A*s + B) written as
    int16, reinterpreted as f16.
  - causal masking via a triangular 0/1 tile multiplied on the GPSIMD
    engine (diagonal blocks only); fully-masked columns skipped in the
    matmuls.
  - o_augT[d+1,sq] += v_aug.T @ expT accumulated over skv tiles in
    PSUM, software-pipelined so the exp stream never waits on a PSUM
    slot.
  - normalization via fast-reciprocal + ones-outer-product broadcast
    matmul, deferred and interleaved into the next chunk's score
    stream; out-projection partialT[Dm,sq] = woT-tile.T @ oT, f16.
"""

import numpy as np

import concourse.bass as bass
import concourse.mybir as mybir
import concourse.tile as tile
from concourse import bacc
from concourse.masks import make_identity
from concourse.bass_utils import run_bass_kernel_spmd

F32 = mybir.dt.float32
F16 = mybir.dt.float16
I16 = mybir.dt.int16

S = 4096        # sequence length
D = 1024        # model dim
HD = 64         # head dim
NH_LOC = 2      # heads per core
DL = HD * NH_LOC  # local dims = 128
SCALE = HD ** -0.5
VW = 80         # padded v_aug row stride (32B-aligned)

N_KC = D // 128       # 8 contraction tiles for projections
N_J = S // 512        # 8 sq chunks
N_I = S // 128        # 32 skv tiles

# Schraudolph f16-bit exp: exp(SCALE*s) ~ f16_from_bits(rne(A*s + B))
SCH_A = 1024.0 / float(np.log(2.0)) * SCALE
SCH_B = 15.0 * 1024.0 - 50.0


def build_kernel(n_cores=8):
    nc = bacc.Bacc("TRN2", target_bir_lowering=False, debug=False,
                   num_devices=n_cores)

    xT = nc.dram_tensor("xT", [D, S], F16, kind="ExternalInput")
    wqT = nc.dram_tensor("wqT", [D, DL], F16, kind="ExternalInput")
    wkT = nc.dram_tensor("wkT", [D, DL], F16, kind="ExternalInput")
    wvT = nc.dram_tensor("wvT", [D, DL], F16, kind="ExternalInput")
    woT = nc.dram_tensor("woT", [DL, D], F16, kind="ExternalInput")
    outT = nc.dram_tensor("outT", [D, S], F16, kind="ExternalOutput")

    with tile.TileContext(nc) as tc:
        build_body(tc, xT, wqT, wkT, wvT, woT, outT)

    nc.compile()
    return nc


def build_body(tc, xT, wqT, wkT, wvT, woT, outT):
    nc = tc.nc

    with tc.tile_pool(name="persist", bufs=1) as persist:
        qT = persist.tile([128, S], F16, tag="qT")
        kT = persist.tile([128, S], F16, tag="kT")
        vT = persist.tile([128, S], F16, tag="vT")
        v_aug = [persist.tile([128, N_I, VW], F16, tag=f"vaug{h}",
                              name=f"vaug{h}") for h in range(2)]
        x_sb = persist.tile([128, N_KC, S], F16, tag="x_sb")
        wq_sb = persist.tile([128, N_KC, DL], F16, tag="wq")
        wk_sb = persist.tile([128, N_KC, DL], F16, tag="wk")
        wv_sb = persist.tile([128, N_KC, DL], F16, tag="wv")
        wo_sb = persist.tile([128, D], F16, tag="wo")
        ones_row = persist.tile([1, HD], F16, tag="ones")
        ident = persist.tile([128, 128], F16, tag="ident")
        tri = persist.tile([128, 128], F16, tag="tri")

        make_identity(nc, ident[:])
        nc.vector.memset(ones_row[:], 1.0)
        nc.vector.memset(v_aug[0][:, :, HD], 1.0)
        nc.vector.memset(v_aug[1][:, :, HD], 1.0)
        # keep (=1.0) where skv partition p <= sq col c
        nc.vector.memset(tri[:], 1.0)
        nc.gpsimd.affine_select(
            out=tri[:], in_=tri[:],
            pattern=[[1, 128]], base=0, channel_multiplier=-1,
            compare_op=mybir.AluOpType.is_ge, fill=0.0,
        )

        with (
            tc.tile_pool(name="sc_ps", bufs=2, space="PSUM") as sc_ps,
            tc.tile_pool(name="pv_ps", bufs=1, space="PSUM") as pv_ps,
            tc.tile_pool(name="pj_ps", bufs=1, space="PSUM") as pj_ps,
            tc.tile_pool(name="op_ps", bufs=1, space="PSUM") as op_ps,
            tc.tile_pool(name="exp_sb", bufs=5) as exp_pool,
            tc.tile_pool(name="att_sb", bufs=4) as att_sb,
            tc.tile_pool(name="out_sb", bufs=6) as out_pool,
        ):
            # DMA issue order: wq first, then the first 512 x columns so
            # q-projection can start ASAP, then the rest.
            for kc in range(N_KC):
                nc.sync.dma_start(out=wq_sb[:, kc, :], in_=wqT[kc * 128:(kc + 1) * 128, :])
            for kc in range(N_KC):
                nc.sync.dma_start(out=x_sb[:, kc, 0:512],
                                  in_=xT[kc * 128:(kc + 1) * 128, 0:512])
            for kc in range(N_KC):
                nc.sync.dma_start(out=wk_sb[:, kc, :], in_=wkT[kc * 128:(kc + 1) * 128, :])
                nc.sync.dma_start(out=wv_sb[:, kc, :], in_=wvT[kc * 128:(kc + 1) * 128, :])
            nc.sync.dma_start(out=wo_sb[:], in_=woT[:])
            for lo, hi in ((512, 1024), (1024, 2560), (2560, S)):
                for kc in range(N_KC):
                    nc.sync.dma_start(out=x_sb[:, kc, lo:hi],
                                      in_=xT[kc * 128:(kc + 1) * 128, lo:hi])

            # warm up the PE (HAM clock gate) while the DMAs land
            warm_ps = op_ps.tile([128, 128], F32, tag="op", name="warm")
            for _ in range(80):
                nc.tensor.matmul(warm_ps[:], ident[:], ident[:],
                                 start=True, stop=True)

            # ---- deferred work items, interleaved into the attention
            # job stream so the PE never idles.  Each item carries a
            # deadline chunk: it MUST be emitted before that chunk's
            # attention stream starts (data dependency). ----
            pending = []

            def drain_pending(k=None):
                n = len(pending) if k is None else min(k, len(pending))
                for _ in range(n):
                    pending.pop(0)[1]()

            def drain_due(j):
                while pending and pending[0][0] <= j:
                    pending.pop(0)[1]()

            def proj_mm(j, which, kc):
                """one contraction step of the q/k/v projection of chunk j
                through the single pj PSUM bank"""
                sl = bass.ts(j, 512)
                w_sb = {"q": wq_sb, "k": wk_sb, "v": wv_sb}[which]
                ps = pj_ps.tile([128, 512], F32, tag="pj", name="pj")
                nc.tensor.matmul(ps[:], w_sb[:, kc, :], x_sb[:, kc, sl],
                                 start=kc == 0, stop=kc == N_KC - 1)
                if kc == N_KC - 1:
                    dst = {"q": qT, "k": kT, "v": vT}[which]
                    nc.vector.tensor_copy(dst[:, sl], ps[:])
                    if which == "v":
                        for ii in range(4):
                            i = 4 * j + ii
                            for h in range(2):
                                nc.sync.dma_start_transpose(
                                    out=v_aug[h][:, i, 0:HD],
                                    in_=vT[64 * h:64 * (h + 1),
                                           128 * i:128 * (i + 1)])

            def queue_proj(j):
                for which in ("q", "k", "v"):
                    for kc in range(N_KC):
                        pending.append(
                            (j, lambda w=which, jj=j, k=kc: proj_mm(jj, w, k)))

            def do_proj(j):
                for which in ("q", "k", "v"):
                    for kc in range(N_KC):
                        proj_mm(j, which, kc)

            def emit_attention(j):
                drain_due(j)
                n_i = 4 * j + 4
                pv = [pv_ps.tile([HD + 1, 512], F32, tag=f"pv{h}",
                                 name=f"pv{h}") for h in range(2)]
                oc = att_sb.tile([128, 512], F16, tag="oc")

                def emit_scores(i):
                    """row-tiled pair: both heads' K=64 matmuls emitted
                    back-to-back on PE row groups 0 / 64"""
                    sc = sc_ps.tile([128, 2, 512], F32, tag="sc", name="sc")
                    rr = i - 4 * j
                    lo = 128 * rr if rr >= 0 else 0
                    for h in range(2):
                        nc.tensor.matmul(
                            sc[:, h, lo:512],
                            kT[64 * h:64 * (h + 1), bass.ts(i, 128)],
                            qT[64 * h:64 * (h + 1), 512 * j + lo:512 * (j + 1)],
                            start=True, stop=True,
                        )
                    ex = exp_pool.tile([128, 2, 512], F16, tag="ex", name="ex")
                    if rr < 0 and i % 4 == 1:
                        # Schraudolph f16-bit exp on DVE (off-diag tiles)
                        nc.vector.tensor_scalar(
                            ex[:].bitcast(I16), sc[:],
                            SCH_A, SCH_B,
                            mybir.AluOpType.mult, mybir.AluOpType.add,
                        )
                    else:
                        nc.scalar.activation(ex[:, :, lo:512], sc[:, :, lo:512],
                                             mybir.ActivationFunctionType.Exp,
                                             scale=SCALE)
                    if rr >= 0:
                        # triangular mask on the diagonal block (GPSIMD)
                        for h in range(2):
                            nc.gpsimd.tensor_mul(
                                ex[:, h, 128 * rr:128 * (rr + 1)],
                                ex[:, h, 128 * rr:128 * (rr + 1)],
                                tri[:])
                    return (i, ex)

                def emit_pv(job):
                    i, ex = job
                    rr = i - 4 * j
                    lo = 128 * rr if rr >= 0 else 0
                    for h in range(2):
                        nc.tensor.matmul(
                            pv[h][:, lo:512],
                            v_aug[h][:, i, 0:HD + 1],
                            ex[:, h, lo:512],
                            start=(i == 0), stop=(i == n_i - 1),
                            skip_group_check=True,
                        )

                prev = None
                for i in range(n_i):
                    job = emit_scores(i)
                    if prev is not None:
                        emit_pv(prev)
                        drain_pending(2)
                    prev = job
                emit_pv(prev)
                if j + 2 < N_J:
                    queue_proj(j + 2)

                # ---- drain the PV psum right away (free the pv slots
                # before chunk j+1's PV matmuls need them) ----
                o_sbs, s_sbs = [], []
                for h in range(2):
                    s_sb = att_sb.tile([1, 512], F32, tag="s_sb", name="s_sb")
                    nc.vector.tensor_copy(s_sb[:], pv[h][HD:HD + 1, :])
                    s_sbs.append(s_sb)
                    o_sb = att_sb.tile([HD, 512], F16, tag="o_sb", name="o_sb")
                    nc.vector.tensor_copy(o_sb[:], pv[h][0:HD, :])
                    o_sbs.append(o_sb)

                # ---- queue the rest of this chunk's tail work ----
                recips = []

                def norm_dve(recips=recips, s_sbs=s_sbs):
                    for h in range(2):
                        recip = att_sb.tile([1, 512], F32, tag="recip",
                                            name="recip")
                        nc.vector.reciprocal_approx_fast(recip[:], s_sbs[h][:])
                        r16 = att_sb.tile([1, 512], F16, tag="recip16",
                                          name="recip16")
                        nc.vector.tensor_copy(r16[:], recip[:])
                        recips.append(r16)

                def norm_mul(oc=oc, recips=recips, o_sbs=o_sbs):
                    for h in range(2):
                        bc = op_ps.tile([HD, 512], F32, tag="op", name="bc")
                        nc.tensor.matmul(bc[:], ones_row[:], recips[h][:],
                                         start=True, stop=True)
                        nc.vector.tensor_mul(oc[h * HD:(h + 1) * HD, :],
                                             o_sbs[h][:], bc[:])

                def outproj(t, j=j, oc=oc):
                    sl = bass.ts(j, 512)
                    op = op_ps.tile([128, 512], F32, tag="op", name="op")
                    nc.tensor.matmul(op[:], wo_sb[:, bass.ts(t, 128)], oc[:],
                                     start=True, stop=True)
                    ot = out_pool.tile([128, 512], F16, tag="ot", name="ot")
                    nc.vector.tensor_copy(ot[:], op[:])
                    nc.sync.dma_start(out=outT[t * 128:(t + 1) * 128, sl],
                                      in_=ot[:])

                pending.append((j + 2, norm_dve))
                pending.append((j + 2, norm_mul))
                pending.append((j + 2, lambda: None))
                for t in range(N_KC):
                    pending.append((j + 2, lambda t=t: outproj(t)))

            do_proj(0)
            do_proj(1)
            for j in range(N_J):
                emit_attention(j)
            drain_pending()


# ---------------- host-side sharding / unsharding ----------------

def shard_inputs(x, w_qkv, w_out, n_cores=8):
    """Full inputs -> per-core in_maps."""
    x2 = np.asarray(x, np.float32).reshape(S, D)
    xT_h = np.ascontiguousarray(x2.T.astype(np.float16))     # [D, S]
    w_qkv = np.asarray(w_qkv, np.float32)
    w_out = np.asarray(w_out, np.float32)
    in_maps = []
    for c in range(n_cores):
        lo, hi = c * DL, (c + 1) * DL
        in_maps.append({
            "xT": xT_h,
            "wqT": np.ascontiguousarray(w_qkv[lo:hi, :].T.astype(np.float16)),
            "wkT": np.ascontiguousarray(w_qkv[D + lo:D + hi, :].T.astype(np.float16)),
            "wvT": np.ascontiguousarray(w_qkv[2 * D + lo:2 * D + hi, :].T.astype(np.float16)),
            "woT": np.ascontiguousarray(w_out[:, lo:hi].T.astype(np.float16)),
        })
    return in_maps


def unshard_outputs(results):
    acc = results[0]["outT"].astype(np.float32)
    for r in results[1:]:
        acc += r["outT"].astype(np.float32)
    return np.ascontiguousarray(acc.T).reshape(1, S, D)


# ---------------- public entry point ----------------

_NC_CACHE = []


def _get_nc():
    if not _NC_CACHE:
        _NC_CACHE.append(build_kernel())
    return _NC_CACHE[0]


def kernel(x, w_qkv, w_out):
    nc = _get_nc()
    in_maps = shard_inputs(x, w_qkv, w_out)
    res = run_bass_kernel_spmd(nc, in_maps, list(range(8)))
    out = unshard_outputs(res.results)
    return out.astype(np.float32)
